# revision 51
# baseline (speedup 1.0000x reference)
"""Linear-chain CRF negative mean log-likelihood on 8 Trainium2 NeuronCores.

Full inputs in, full (scalar) output out. Data-parallel over the batch:
each core processes B/8 = 1024 sequences end-to-end:

  - emission scores em[b,t,l] = feat_x @ W.T  via PE matmuls (x transposed
    on-chip with PE transpose-mode, bf16)
  - partition function via the forward algorithm run in scaled-exp space:
    A_t = (expTr'.T @ A_{t-1}) * exp(em_t)  -- one full-K PE matmul per step
    with the constant per-step scale e^{-c} folded into expTr' = exp(Tr - c);
    logZ = log(sum A_T) + T*c
  - gold emission score: St[d,l] = sum_{b,t: y=l} x[b,t,d] accumulated with
    x-stationary PE matmuls (moving operand = one-hot, 26 columns only),
    em_score = <Wt, St>
  - gold transition score via count matrix C[l,l'] = sum oh_t.T oh_{t+1},
    tr_score = <Tr, C>
  - the final forward messages A_T ship to HBM raw; the host finishes
    logZ = ln(sum_l A_T) + T*c in f64, keeping the Ln off the device's
    serial tail

The loop is software-pipelined two steps deep: iteration t runs the
em matmuls / exp / gold matmuls / DP step for t-2 and the transposes
for t, so every cross-engine dependency (PE -> Act exp -> DVE mult ->
PE DP) has a full iteration of slack. The PSUM->SBUF copy of the
transposed x is split between DVE and Act to balance engine load, and
a short train of dummy transposes warms the PE clock ramp while the
first x block is still in flight.

Each core writes partial sums; the host combines them into the scalar loss.
"""

import numpy as np

L = 26
D = 128
T = 64
B = 8192
NCORES = 8
BC = B // NCORES  # 1024 sequences per core

# Constant per-step scale for the exp-space forward DP (replaces a per-step
# schedule; the partial sums of the true per-step log increments stay within
# ~±10 of t*C_CONST, well inside fp32 range). Added back to logZ on the host.
C_CONST = 4.04

_CACHE: dict = {}
TRACE = False  # set by test harness to capture NTFF profile / exec time

# Instruction opcodes whose hardware structs tolerate multiple sync waits (or
# that walrus lowers specially). Everything else gets excess waits peeled onto
# EventSemaphore instructions inserted just before it (same engine).
_MULTIWAIT_OK = {
    "Call",
    "UnconditionalBranch",
    "ConditionalBranch",
}


def _legalize_waits(bir_bytes: bytes) -> bytes:
    """Split >1 sync waits per compute instruction into EventSemaphore preludes.

    The TRN2 64-byte instruction structs hold a single sync-wait command;
    Tile attaches multi-engine waits directly, which walrus codegen rejects
    ("Too many sync wait commands"). Peeling extra waits onto same-engine
    EventSemaphore instructions placed immediately before is semantically
    identical (engine streams execute in order).
    """
    import json

    d = json.loads(bir_bytes)
    n = 0
    for fn in d["functions"]:
        for blk in fn["blocks"]:
            out = []
            for inst in blk["instructions"]:
                si = inst.get("sync_info")
                if (
                    si
                    and len(si.get("on_wait", [])) > 1
                    and inst["opcode"] not in _MULTIWAIT_OK
                ):
                    waits = si["on_wait"]
                    for w in waits[:-1]:
                        n += 1
                        out.append({
                            "debug": inst.get("debug", 0),
                            "engine": inst["engine"],
                            "ins": [],
                            "name": f"wsplit-{n}-{inst['name']}",
                            "opcode": "EventSemaphore",
                            "outs": [],
                            "sync_info": {"on_update": [], "on_wait": [w]},
                        })
                    si["on_wait"] = [waits[-1]]
                out.append(inst)
            blk["instructions"] = out
    return json.dumps(d).encode()


def build_program():
    """Build the per-core Bass/Tile program (identical SPMD program)."""
    from contextlib import ExitStack

    import concourse.bass as bass
    import concourse.tile as tile
    from concourse import mybir
    from concourse.masks import make_identity

    f32 = mybir.dt.float32
    f32r = mybir.dt.float32r
    bf16 = mybir.dt.bfloat16
    i32 = mybir.dt.int32
    AF = mybir.ActivationFunctionType
    OP = mybir.AluOpType

    nc = bass.Bass("TRN2", target_bir_lowering=False, debug=False)

    x_d = nc.dram_tensor("x", [BC, T, D], f32, kind="ExternalInput").ap()
    y_d = nc.dram_tensor("y", [BC, T], i32, kind="ExternalInput").ap()
    p_d = nc.dram_tensor("p", [L * D + L * L], f32, kind="ExternalInput").ap()
    # single output: cols 0:256 = final forward messages A_T, col 256 =
    # per-partition em-score partials, col 257 = tr-score partials
    out_d = nc.dram_tensor("out", [128, 260], f32, kind="ExternalOutput").ap()

    # views: partition p <- b % 128, so per-t tiles are [128 b, ...]
    yv = y_d.rearrange("(c p) t -> p c t", p=128)       # [128, 8, 64]
    Wv = p_d[: L * D].rearrange("(l d) -> l d", l=L)
    Trv = p_d[L * D :].rearrange("(a b) -> a b", a=L)

    # (start, len) DMA blocks covering t=0..T-1
    XPLAN = _CACHE.get("XPLAN")
    if XPLAN is None:
        XPLAN = [(2 * i, 2) for i in range(5)] + [
            (10 + 4 * q, 4) for q in range((T - 10) // 4)
        ] + [(62, 2)]
    BLOCK_AHEAD = _CACHE.get("BLOCK_AHEAD", 4)  # blocks issued pre-loop

    with ExitStack() as ctx:
        tc = ctx.enter_context(tile.TileContext(nc))

        const = ctx.enter_context(tc.tile_pool(name="const", bufs=1))
        from collections import Counter
        _sizes = Counter(n for _, n in XPLAN)
        xbpool = {
            n: ctx.enter_context(tc.tile_pool(name=f"xbpool{n}", bufs=cnt))
            for n, cnt in _sizes.items()
        }
        ohpool = ctx.enter_context(tc.tile_pool(name="ohpool", bufs=5))
        xtpool = ctx.enter_context(tc.tile_pool(name="xtpool", bufs=3))
        eempool = ctx.enter_context(tc.tile_pool(name="eempool", bufs=4))
        apool = ctx.enter_context(tc.tile_pool(name="apool", bufs=4))
        fpool = ctx.enter_context(tc.tile_pool(name="fpool", bufs=1))
        ps_xt = ctx.enter_context(tc.tile_pool(name="ps_xt", bufs=3, space="PSUM"))
        ps_em = ctx.enter_context(tc.tile_pool(name="ps_em", bufs=3, space="PSUM"))
        ps_u = ctx.enter_context(tc.tile_pool(name="ps_u", bufs=1, space="PSUM"))
        ps_acc = ctx.enter_context(tc.tile_pool(name="ps_acc", bufs=1, space="PSUM"))

        # ---- Pool-engine setup FIRST: the x DMAs below occupy the in-order
        # Pool queue for ~30us of descriptor generation, so anything Pool
        # must produce (identity for PE transposes, iota) goes before them ----
        # identity: zero on DVE (keeps the serial Pool path short); the
        # diagonal fill and iota are emitted after the first x-block DMAs so
        # the scheduler gives descriptor generation the Pool queue first
        ident = const.tile([128, 128], bf16)
        nc.vector.memset(ident, 0.0)

        iota26 = const.tile([128, 26], i32)
        iotaexp = const.tile([128, 26, 8], bf16)
        y_bf = const.tile([128, T, 8], bf16)

        # ---- input DMAs. x goes through gpsimd/SWDGE (the only engine that
        # can cast f32->bf16 in the DGE); y/W/Tr ride the sync-engine HWDGE
        # path in parallel, y first since oh-generation needs it earliest ----
        y_sb = const.tile([128, 8, T], i32)
        nc.sync.dma_start(out=y_sb[:, :, 0 : T // 4], in_=yv[:, :, 0 : T // 4])

        W_sb = const.tile([26, 128], f32)
        nc.sync.dma_start(out=W_sb, in_=Wv)

        # exp(Tr - c) staged per partition-group for the block-diagonal DP
        # operand (activation lanes are partition-aligned, so each group gets
        # its own copy of Tr at its partition offset)
        Trstage = const.tile([128, 26], f32)
        for g in range(4):
            nc.sync.dma_start(out=Trstage[32 * g : 32 * g + 26, :], in_=Trv)

        Tr_sb = const.tile([26, 26], f32)
        nc.sync.dma_start(out=Tr_sb, in_=Trv)

        # x block plan: leading 2-step blocks let the PE start early; the
        # steady state uses 4-step blocks (2KB HBM runs, cheap SWDGE
        # descgen per timestep). Issued lazily with BLOCK_AHEAD blocks of
        # lookahead so Pool descriptor generation paces with consumption
        # instead of monopolizing the in-order Pool queue up front.
        xblocks = []  # list of (t_start, nsteps, tile)

        def issue_block():
            i = len(xblocks)
            if i >= len(XPLAN):
                return
            s, n = XPLAN[i]
            # flat [128, 1024n] tile: the whole per-partition region is one
            # contiguous run, so SWDGE descgen sees the largest element size
            xb = xbpool[n].tile([128, 1024 * n], bf16, tag=f"xb{n}", name=f"xb{s}")
            xin = x_d[:, s : s + n].rearrange("(c p) t d -> p c (t d)", p=128)
            nc.gpsimd.dma_start(
                out=xb.rearrange("p (c r) -> p c r", c=8), in_=xin
            )
            xblocks.append((s, n, xb))

        make_identity(nc, ident, nomemset=True)
        nc.gpsimd.iota(iota26, pattern=[[1, 26]], base=0, channel_multiplier=0)
        for _ in range(BLOCK_AHEAD):
            issue_block()

        nc.vector.tensor_copy(
            iotaexp, iota26.rearrange("p l -> p l ()").broadcast_to([128, 26, 8])
        )
        # y staged as bf16 t-major so the per-step one-hot compare runs in
        # the DVE 2x packed mode (label values 0..25 are exact in bf16).
        # Converted in two chunks tracking the split y DMA arrivals.
        nc.vector.tensor_copy(
            y_bf[:, 0 : T // 4], y_sb[:, :, 0 : T // 4].rearrange("p c t -> p t c")
        )

        def x_slice(t, c):
            """SBUF view of x[t] chunk c: [128 b, 128 d] bf16."""
            for s, n, xb in xblocks:
                if s <= t < s + n:
                    o = c * 128 * n + 128 * (t - s)
                    return xb[:, o : o + 128]
            raise KeyError(t)

        # ---- constants ----
        negc = const.tile([128, 1], f32)
        nc.vector.memset(negc, -C_CONST)

        # expTr' = exp(Tr - c) as a block-diagonal [128, 128] (4 copies along
        # the diagonal) so the whole 4-group DP step is ONE full-K matmul
        expTr = const.tile([128, 128], f32r)
        nc.vector.memset(expTr.bitcast(f32), 0.0)
        for g in range(4):
            nc.scalar.activation(
                expTr[32 * g : 32 * g + 26, 32 * g : 32 * g + 26],
                Trstage[32 * g : 32 * g + 26, :],
                AF.Exp,
                bias=negc[32 * g : 32 * g + 26],
            )

        # combined output tile: A_T lands in cols 0:256 via the final DP
        # multiply; gold-score reduces fill cols 256:258; one DMA ships all
        comb = const.tile([128, 260], f32)
        nc.vector.memset(comb[:, 256:260], 0.0)

        NWARM = _CACHE.get("NWARM", 14)
        if NWARM:
            warm_ps = ps_xt.tile([128, 1024], bf16, tag="xt", name="warm")
            for _ in range(NWARM):
                nc.tensor.transpose(warm_ps[0:64, 0:128], ident[:, 0:64], ident)

        # persistent psum accumulators for the gold scores, sharing one
        # PSUM bank (both are tiny; banks are the scarce resource)
        acc = ps_acc.tile([128, 64], f32)
        St_ps = acc[:, 0:26]
        C_ps = acc[0:26, 32:58]
        nc.vector.memset(St_ps, 0.0)
        nc.vector.memset(C_ps, 0.0)

        # ---- software-pipelined main loop ----
        # iteration t emits: transposes(t); em(t-1); S(t-2); C(t-2,t-1);
        # DP matmul u(t-1); oh(t) [DVE]; copies(t) [DVE/Act/Pool];
        # exp(t-1) [Act]; A(t-1) mult [DVE].
        W_bf = const.tile([26, 128], bf16)
        Wt_bf = const.tile([128, 32], bf16)
        Wt_gold = const.tile([128, 26], f32)

        oh = {}
        xt_sb = {}
        em_ps = {}
        eem = {}
        A = {}

        def emit_transposes(t):
            xt_p = ps_xt.tile([128, 1024], bf16, tag="xt", name=f"xtp{t}")
            for c in range(8):
                nc.tensor.transpose(
                    xt_p[:, 128 * c : 128 * (c + 1)], x_slice(t, c), ident
                )
            return xt_p

        def emit_copies(t, xt_p):
            xt_s = xtpool.tile([128, 1024], bf16, tag="xts", name=f"xts{t}")
            # split exactly on an em-group boundary so each em matmul waits
            # on a single producer semaphore
            nc.vector.tensor_copy(xt_s[:, 0:768], xt_p[:, 0:768])
            nc.scalar.copy(xt_s[:, 768:1024], xt_p[:, 768:1024])
            xt_sb[t] = xt_s

        def emit_oh(t):
            oh_t = ohpool.tile([128, 26, 8], bf16, tag="oh", name=f"oh{t}")
            nc.vector.tensor_tensor(
                out=oh_t,
                in0=y_bf[:, t : t + 1, :].broadcast_to([128, 26, 8]),
                in1=iotaexp,
                op=OP.is_equal,
            )
            oh[t] = oh_t

        def emit_em(t):
            e_ps = ps_em.tile([128, 256], f32, tag="em", name=f"em{t}")
            for g in range(4):
                nc.tensor.matmul(
                    e_ps[32 * g : 32 * (g + 1), :],
                    lhsT=Wt_bf,
                    rhs=xt_sb[t][:, 256 * g : 256 * (g + 1)],
                    start=True,
                    stop=True,
                    tile_position=(0, 32 * g),
                )
            del xt_sb[t]
            em_ps[t] = e_ps

        def emit_exp(t):
            # t=0 becomes A_0 = exp(em_0 - c) directly
            if t == 0:
                dst = apool.tile([128, 256], f32r, tag="A", name="A0")
                nc.scalar.activation(dst, em_ps[t], AF.Exp, bias=negc)
                A[t] = dst
            else:
                dst = eempool.tile([128, 256], f32, tag="eem", name=f"eem{t}")
                nc.scalar.activation(dst, em_ps[t], AF.Exp)
                eem[t] = dst
            del em_ps[t]

        def emit_gold(t):
            # St[d, l] += x_t[c].T @ oh_t[c]  (x stationary, 26 moving cols)
            for c in range(8):
                nc.tensor.matmul(
                    St_ps,
                    lhsT=x_slice(t, c),
                    rhs=oh[t][:, :, c],
                    start=False,
                    stop=False,
                    skip_group_check=True,
                )

        def emit_count(t):
            # C[l, l'] += oh_t[c].T @ oh_{t+1}[c]
            for c in range(8):
                nc.tensor.matmul(
                    C_ps,
                    lhsT=oh[t][:, :, c],
                    rhs=oh[t + 1][:, :, c],
                    start=False,
                    stop=False,
                    skip_group_check=True,
                )

        u = {}

        def emit_u(t):
            # u_t = expTr'.T @ A_{t-1}
            u_ps = ps_u.tile([128, 256], f32, tag="u", name=f"u{t}")
            nc.tensor.matmul(u_ps, lhsT=expTr, rhs=A[t - 1], start=True, stop=True)
            del A[t - 1]
            u[t] = u_ps

        def emit_mult(t):
            # A_t = u_t * exp(em_t)
            A_t = apool.tile([128, 256], f32r, tag="A", name=f"A{t}")
            nc.vector.tensor_mul(A_t, u[t], eem[t])
            del u[t], eem[t]
            A[t] = A_t

        for t in range(T):
            emit_oh(t)
            if t == 2:
                nc.sync.dma_start(
                    out=y_sb[:, :, T // 4 : T], in_=yv[:, :, T // 4 : T]
                )
            if t == 3:
                nc.vector.tensor_copy(
                    y_bf[:, T // 4 : T],
                    y_sb[:, :, T // 4 : T].rearrange("p c t -> p t c"),
                )
            if len(xblocks) < len(XPLAN) and t >= xblocks[-1][0]:
                issue_block()
            if t >= 2:
                emit_em(t - 2)
            xt_p = emit_transposes(t)
            if t == 0:
                # W transpose setup rides behind the first transposes so the
                # PE never head-of-line blocks on the W DMA
                nc.vector.tensor_copy(W_bf, W_sb)
                wt_ps = ps_em.tile([128, 26], bf16, tag="em", name="wt")
                nc.tensor.transpose(wt_ps, W_bf, ident[0:26, 0:26])
                nc.vector.memset(Wt_bf, 0.0)
                nc.vector.tensor_copy(Wt_bf[:, 0:26], wt_ps)
                nc.vector.tensor_copy(Wt_gold, wt_ps)
            if t >= 2:
                emit_gold(t - 2)
                emit_count(t - 2)
                emit_exp(t - 2)
            if t >= 3:
                emit_u(t - 2)
                emit_mult(t - 2)
            if t == T - 1:
                emit_em(t - 1)
                emit_exp(t - 1)
                emit_u(t - 1)
                emit_mult(t - 1)
            emit_copies(t, xt_p)

        # ---- epilogue: drain the pipeline (the T-2 DP step was pulled
        # into the last loop iteration). The final multiply writes straight
        # into the output tile.
        emit_em(T - 1)
        emit_exp(T - 1)
        emit_gold(T - 2)
        emit_count(T - 2)
        emit_u(T - 1)
        nc.vector.tensor_mul(
            comb[:, 0:256].bitcast(f32), u[T - 1], eem[T - 1]
        )
        emit_gold(T - 1)

        # ---- finale ----
        # em_score = <Wt, St>, tr_score = <Tr, C>
        Sw = fpool.tile([128, 26], f32)
        nc.vector.tensor_mul(Sw, St_ps, Wt_gold)
        nc.vector.tensor_reduce(
            out=comb[:, 256:257], in_=Sw, axis=mybir.AxisListType.X, op=OP.add
        )
        Cw = fpool.tile([26, 26], f32)
        nc.vector.tensor_mul(Cw, C_ps, Tr_sb)
        nc.vector.tensor_reduce(
            out=comb[0:26, 257:258], in_=Cw, axis=mybir.AxisListType.X, op=OP.add
        )

        nc.sync.dma_start(out=out_d, in_=comb)

    fixed = _legalize_waits(nc.to_json_bytes())
    nc.to_json_bytes = lambda: fixed  # shadow for all compile paths
    return nc


def kernel(feat_x: np.ndarray, input_y: np.ndarray, params: np.ndarray) -> np.ndarray:
    from concourse.bass_utils import run_bass_kernel_spmd

    if "nc" not in _CACHE:
        _CACHE["nc"] = build_program()
    nc = _CACHE["nc"]

    feat_x = np.ascontiguousarray(feat_x, dtype=np.float32)
    input_y = np.ascontiguousarray(input_y, dtype=np.int32)
    params = np.ascontiguousarray(params, dtype=np.float32)

    in_maps = []
    for m in range(NCORES):
        sl = slice(m * BC, (m + 1) * BC)
        in_maps.append({"x": feat_x[sl], "y": input_y[sl], "p": params})

    res = run_bass_kernel_spmd(
        nc, in_maps, core_ids=list(range(NCORES)), trace=TRACE
    )
    _CACHE["last_results"] = res

    em_sum = tr_sum = lz_sum = 0.0
    for m in range(NCORES):
        out = res.results[m]["out"].astype(np.float64)
        em_sum += out[:, 256].sum()
        tr_sum += out[:, 257].sum()
        for g in range(4):
            lz_sum += np.log(out[32 * g : 32 * g + 26, 0:256].sum(axis=0)).sum()
    lz_sum += B * T * C_CONST
    loss = -(em_sum + tr_sum - lz_sum) / B
    return np.float32(loss)


# revision 52
# speedup vs baseline: 1.0008x; 1.0008x over previous
"""Linear-chain CRF negative mean log-likelihood on 8 Trainium2 NeuronCores.

Full inputs in, full (scalar) output out. Data-parallel over the batch:
each core processes B/8 = 1024 sequences end-to-end:

  - emission scores em[b,t,l] = feat_x @ W.T  via PE matmuls (x transposed
    on-chip with PE transpose-mode, bf16)
  - partition function via the forward algorithm run in scaled-exp space:
    A_t = (expTr'.T @ A_{t-1}) * exp(em_t)  -- one full-K PE matmul per step
    with the constant per-step scale e^{-c} folded into expTr' = exp(Tr - c);
    logZ = log(sum A_T) + T*c
  - gold emission score: St[d,l] = sum_{b,t: y=l} x[b,t,d] accumulated with
    x-stationary PE matmuls (moving operand = one-hot, 26 columns only),
    em_score = <Wt, St>
  - gold transition score via count matrix C[l,l'] = sum oh_t.T oh_{t+1},
    tr_score = <Tr, C>
  - the final forward messages A_T ship to HBM raw; the host finishes
    logZ = ln(sum_l A_T) + T*c in f64, keeping the Ln off the device's
    serial tail

The loop is software-pipelined two steps deep: iteration t runs the
em matmuls / exp / gold matmuls / DP step for t-2 and the transposes
for t, so every cross-engine dependency (PE -> Act exp -> DVE mult ->
PE DP) has a full iteration of slack. The PSUM->SBUF copy of the
transposed x is split between DVE and Act to balance engine load, and
a short train of dummy transposes warms the PE clock ramp while the
first x block is still in flight.

Each core writes partial sums; the host combines them into the scalar loss.
"""

import numpy as np

L = 26
D = 128
T = 64
B = 8192
NCORES = 8
BC = B // NCORES  # 1024 sequences per core

# Constant per-step scale for the exp-space forward DP (replaces a per-step
# schedule; the partial sums of the true per-step log increments stay within
# ~±10 of t*C_CONST, well inside fp32 range). Added back to logZ on the host.
C_CONST = 4.04

_CACHE: dict = {}
TRACE = False  # set by test harness to capture NTFF profile / exec time

# Instruction opcodes whose hardware structs tolerate multiple sync waits (or
# that walrus lowers specially). Everything else gets excess waits peeled onto
# EventSemaphore instructions inserted just before it (same engine).
_MULTIWAIT_OK = {
    "Call",
    "UnconditionalBranch",
    "ConditionalBranch",
}


def _legalize_waits(bir_bytes: bytes) -> bytes:
    """Split >1 sync waits per compute instruction into EventSemaphore preludes.

    The TRN2 64-byte instruction structs hold a single sync-wait command;
    Tile attaches multi-engine waits directly, which walrus codegen rejects
    ("Too many sync wait commands"). Peeling extra waits onto same-engine
    EventSemaphore instructions placed immediately before is semantically
    identical (engine streams execute in order).
    """
    import json

    d = json.loads(bir_bytes)
    n = 0
    for fn in d["functions"]:
        for blk in fn["blocks"]:
            out = []
            for inst in blk["instructions"]:
                si = inst.get("sync_info")
                if (
                    si
                    and len(si.get("on_wait", [])) > 1
                    and inst["opcode"] not in _MULTIWAIT_OK
                ):
                    waits = si["on_wait"]
                    for w in waits[:-1]:
                        n += 1
                        out.append({
                            "debug": inst.get("debug", 0),
                            "engine": inst["engine"],
                            "ins": [],
                            "name": f"wsplit-{n}-{inst['name']}",
                            "opcode": "EventSemaphore",
                            "outs": [],
                            "sync_info": {"on_update": [], "on_wait": [w]},
                        })
                    si["on_wait"] = [waits[-1]]
                out.append(inst)
            blk["instructions"] = out
    return json.dumps(d).encode()


def build_program():
    """Build the per-core Bass/Tile program (identical SPMD program)."""
    from contextlib import ExitStack

    import concourse.bass as bass
    import concourse.tile as tile
    from concourse import mybir
    from concourse.masks import make_identity

    f32 = mybir.dt.float32
    f32r = mybir.dt.float32r
    bf16 = mybir.dt.bfloat16
    i32 = mybir.dt.int32
    AF = mybir.ActivationFunctionType
    OP = mybir.AluOpType

    nc = bass.Bass("TRN2", target_bir_lowering=False, debug=False)

    x_d = nc.dram_tensor("x", [BC, T, D], f32, kind="ExternalInput").ap()
    y_d = nc.dram_tensor("y", [BC, T], i32, kind="ExternalInput").ap()
    p_d = nc.dram_tensor("p", [L * D + L * L], f32, kind="ExternalInput").ap()
    # single output: cols 0:256 = final forward messages A_T, col 256 =
    # per-partition em-score partials, col 257 = tr-score partials
    out_d = nc.dram_tensor("out", [128, 260], f32, kind="ExternalOutput").ap()

    # views: partition p <- b % 128, so per-t tiles are [128 b, ...]
    yv = y_d.rearrange("(c p) t -> p c t", p=128)       # [128, 8, 64]
    Wv = p_d[: L * D].rearrange("(l d) -> l d", l=L)
    Trv = p_d[L * D :].rearrange("(a b) -> a b", a=L)

    # (start, len) DMA blocks covering t=0..T-1
    XPLAN = _CACHE.get("XPLAN")
    if XPLAN is None:
        XPLAN = [(2 * i, 2) for i in range(5)] + [
            (10 + 4 * q, 4) for q in range((T - 10) // 4)
        ] + [(62, 2)]
    BLOCK_AHEAD = _CACHE.get("BLOCK_AHEAD", 4)  # blocks issued pre-loop

    with ExitStack() as ctx:
        tc = ctx.enter_context(tile.TileContext(nc))

        const = ctx.enter_context(tc.tile_pool(name="const", bufs=1))
        from collections import Counter
        _sizes = Counter(n for _, n in XPLAN)
        xbpool = {
            n: ctx.enter_context(tc.tile_pool(name=f"xbpool{n}", bufs=cnt))
            for n, cnt in _sizes.items()
        }
        ohpool = ctx.enter_context(tc.tile_pool(name="ohpool", bufs=5))
        xtpool = ctx.enter_context(tc.tile_pool(name="xtpool", bufs=3))
        eempool = ctx.enter_context(tc.tile_pool(name="eempool", bufs=4))
        apool = ctx.enter_context(tc.tile_pool(name="apool", bufs=4))
        fpool = ctx.enter_context(tc.tile_pool(name="fpool", bufs=1))
        ps_xt = ctx.enter_context(tc.tile_pool(name="ps_xt", bufs=3, space="PSUM"))
        ps_em = ctx.enter_context(tc.tile_pool(name="ps_em", bufs=3, space="PSUM"))
        ps_u = ctx.enter_context(tc.tile_pool(name="ps_u", bufs=1, space="PSUM"))
        ps_acc = ctx.enter_context(tc.tile_pool(name="ps_acc", bufs=1, space="PSUM"))

        # ---- Pool-engine setup FIRST: the x DMAs below occupy the in-order
        # Pool queue for ~30us of descriptor generation, so anything Pool
        # must produce (identity for PE transposes, iota) goes before them ----
        # identity: zero on DVE (keeps the serial Pool path short); the
        # diagonal fill and iota are emitted after the first x-block DMAs so
        # the scheduler gives descriptor generation the Pool queue first
        ident = const.tile([128, 128], bf16)
        nc.vector.memset(ident, 0.0)

        iota26 = const.tile([128, 26], i32)
        iotaexp = const.tile([128, 26, 8], bf16)
        y_bf = const.tile([128, T, 8], bf16)

        # ---- input DMAs. x goes through gpsimd/SWDGE (the only engine that
        # can cast f32->bf16 in the DGE); y/W/Tr ride the sync-engine HWDGE
        # path in parallel, y first since oh-generation needs it earliest ----
        y_sb = const.tile([128, 8, T], i32)
        nc.sync.dma_start(out=y_sb[:, :, 0 : T // 8], in_=yv[:, :, 0 : T // 8])

        W_sb = const.tile([26, 128], f32)
        nc.sync.dma_start(out=W_sb, in_=Wv)

        # exp(Tr - c) staged per partition-group for the block-diagonal DP
        # operand (activation lanes are partition-aligned, so each group gets
        # its own copy of Tr at its partition offset)
        Trstage = const.tile([128, 26], f32)
        for g in range(4):
            nc.sync.dma_start(out=Trstage[32 * g : 32 * g + 26, :], in_=Trv)

        Tr_sb = const.tile([26, 26], f32)
        nc.sync.dma_start(out=Tr_sb, in_=Trv)

        # x block plan: leading 2-step blocks let the PE start early; the
        # steady state uses 4-step blocks (2KB HBM runs, cheap SWDGE
        # descgen per timestep). Issued lazily with BLOCK_AHEAD blocks of
        # lookahead so Pool descriptor generation paces with consumption
        # instead of monopolizing the in-order Pool queue up front.
        xblocks = []  # list of (t_start, nsteps, tile)

        def issue_block():
            i = len(xblocks)
            if i >= len(XPLAN):
                return
            s, n = XPLAN[i]
            # flat [128, 1024n] tile: the whole per-partition region is one
            # contiguous run, so SWDGE descgen sees the largest element size
            xb = xbpool[n].tile([128, 1024 * n], bf16, tag=f"xb{n}", name=f"xb{s}")
            xin = x_d[:, s : s + n].rearrange("(c p) t d -> p c (t d)", p=128)
            nc.gpsimd.dma_start(
                out=xb.rearrange("p (c r) -> p c r", c=8), in_=xin
            )
            xblocks.append((s, n, xb))

        make_identity(nc, ident, nomemset=True)
        nc.gpsimd.iota(iota26, pattern=[[1, 26]], base=0, channel_multiplier=0)
        for _ in range(BLOCK_AHEAD):
            issue_block()

        nc.vector.tensor_copy(
            iotaexp, iota26.rearrange("p l -> p l ()").broadcast_to([128, 26, 8])
        )
        # y staged as bf16 t-major so the per-step one-hot compare runs in
        # the DVE 2x packed mode (label values 0..25 are exact in bf16).
        # Converted in two chunks tracking the split y DMA arrivals.
        nc.vector.tensor_copy(
            y_bf[:, 0 : T // 8], y_sb[:, :, 0 : T // 8].rearrange("p c t -> p t c")
        )

        def x_slice(t, c):
            """SBUF view of x[t] chunk c: [128 b, 128 d] bf16."""
            for s, n, xb in xblocks:
                if s <= t < s + n:
                    o = c * 128 * n + 128 * (t - s)
                    return xb[:, o : o + 128]
            raise KeyError(t)

        # ---- constants ----
        negc = const.tile([128, 1], f32)
        nc.vector.memset(negc, -C_CONST)

        # expTr' = exp(Tr - c) as a block-diagonal [128, 128] (4 copies along
        # the diagonal) so the whole 4-group DP step is ONE full-K matmul
        expTr = const.tile([128, 128], f32r)
        nc.vector.memset(expTr.bitcast(f32), 0.0)
        for g in range(4):
            nc.scalar.activation(
                expTr[32 * g : 32 * g + 26, 32 * g : 32 * g + 26],
                Trstage[32 * g : 32 * g + 26, :],
                AF.Exp,
                bias=negc[32 * g : 32 * g + 26],
            )

        # combined output tile: A_T lands in cols 0:256 via the final DP
        # multiply; gold-score reduces fill cols 256:258; one DMA ships all
        comb = const.tile([128, 260], f32)
        nc.vector.memset(comb[:, 256:260], 0.0)

        NWARM = _CACHE.get("NWARM", 14)
        if NWARM:
            warm_ps = ps_xt.tile([128, 1024], bf16, tag="xt", name="warm")
            for _ in range(NWARM):
                nc.tensor.transpose(warm_ps[0:64, 0:128], ident[:, 0:64], ident)

        # persistent psum accumulators for the gold scores, sharing one
        # PSUM bank (both are tiny; banks are the scarce resource)
        acc = ps_acc.tile([128, 64], f32)
        St_ps = acc[:, 0:26]
        C_ps = acc[0:26, 32:58]
        nc.vector.memset(St_ps, 0.0)
        nc.vector.memset(C_ps, 0.0)

        # ---- software-pipelined main loop ----
        # iteration t emits: transposes(t); em(t-1); S(t-2); C(t-2,t-1);
        # DP matmul u(t-1); oh(t) [DVE]; copies(t) [DVE/Act/Pool];
        # exp(t-1) [Act]; A(t-1) mult [DVE].
        W_bf = const.tile([26, 128], bf16)
        Wt_bf = const.tile([128, 32], bf16)
        Wt_gold = const.tile([128, 26], f32)

        oh = {}
        xt_sb = {}
        em_ps = {}
        eem = {}
        A = {}

        def emit_transposes(t):
            xt_p = ps_xt.tile([128, 1024], bf16, tag="xt", name=f"xtp{t}")
            for c in range(8):
                nc.tensor.transpose(
                    xt_p[:, 128 * c : 128 * (c + 1)], x_slice(t, c), ident
                )
            return xt_p

        def emit_copies(t, xt_p):
            xt_s = xtpool.tile([128, 1024], bf16, tag="xts", name=f"xts{t}")
            # split exactly on an em-group boundary so each em matmul waits
            # on a single producer semaphore
            nc.vector.tensor_copy(xt_s[:, 0:768], xt_p[:, 0:768])
            nc.scalar.copy(xt_s[:, 768:1024], xt_p[:, 768:1024])
            xt_sb[t] = xt_s

        def emit_oh(t):
            oh_t = ohpool.tile([128, 26, 8], bf16, tag="oh", name=f"oh{t}")
            nc.vector.tensor_tensor(
                out=oh_t,
                in0=y_bf[:, t : t + 1, :].broadcast_to([128, 26, 8]),
                in1=iotaexp,
                op=OP.is_equal,
            )
            oh[t] = oh_t

        def emit_em(t):
            e_ps = ps_em.tile([128, 256], f32, tag="em", name=f"em{t}")
            for g in range(4):
                nc.tensor.matmul(
                    e_ps[32 * g : 32 * (g + 1), :],
                    lhsT=Wt_bf,
                    rhs=xt_sb[t][:, 256 * g : 256 * (g + 1)],
                    start=True,
                    stop=True,
                    tile_position=(0, 32 * g),
                )
            del xt_sb[t]
            em_ps[t] = e_ps

        def emit_exp(t):
            # t=0 becomes A_0 = exp(em_0 - c) directly
            if t == 0:
                dst = apool.tile([128, 256], f32r, tag="A", name="A0")
                nc.scalar.activation(dst, em_ps[t], AF.Exp, bias=negc)
                A[t] = dst
            else:
                dst = eempool.tile([128, 256], f32, tag="eem", name=f"eem{t}")
                nc.scalar.activation(dst, em_ps[t], AF.Exp)
                eem[t] = dst
            del em_ps[t]

        def emit_gold(t):
            # St[d, l] += x_t[c].T @ oh_t[c]  (x stationary, 26 moving cols)
            for c in range(8):
                nc.tensor.matmul(
                    St_ps,
                    lhsT=x_slice(t, c),
                    rhs=oh[t][:, :, c],
                    start=False,
                    stop=False,
                    skip_group_check=True,
                )

        def emit_count(t):
            # C[l, l'] += oh_t[c].T @ oh_{t+1}[c]
            for c in range(8):
                nc.tensor.matmul(
                    C_ps,
                    lhsT=oh[t][:, :, c],
                    rhs=oh[t + 1][:, :, c],
                    start=False,
                    stop=False,
                    skip_group_check=True,
                )

        u = {}

        def emit_u(t):
            # u_t = expTr'.T @ A_{t-1}
            u_ps = ps_u.tile([128, 256], f32, tag="u", name=f"u{t}")
            nc.tensor.matmul(u_ps, lhsT=expTr, rhs=A[t - 1], start=True, stop=True)
            del A[t - 1]
            u[t] = u_ps

        def emit_mult(t):
            # A_t = u_t * exp(em_t)
            A_t = apool.tile([128, 256], f32r, tag="A", name=f"A{t}")
            nc.vector.tensor_mul(A_t, u[t], eem[t])
            del u[t], eem[t]
            A[t] = A_t

        for t in range(T):
            emit_oh(t)
            if t == 1:
                nc.sync.dma_start(
                    out=y_sb[:, :, T // 8 : T], in_=yv[:, :, T // 8 : T]
                )
            if t == 3:
                nc.vector.tensor_copy(
                    y_bf[:, T // 8 : T],
                    y_sb[:, :, T // 8 : T].rearrange("p c t -> p t c"),
                )
            if len(xblocks) < len(XPLAN) and t >= xblocks[-1][0]:
                issue_block()
            if t >= 2:
                emit_em(t - 2)
            xt_p = emit_transposes(t)
            if t == 0:
                # W transpose setup rides behind the first transposes so the
                # PE never head-of-line blocks on the W DMA
                nc.vector.tensor_copy(W_bf, W_sb)
                wt_ps = ps_em.tile([128, 26], bf16, tag="em", name="wt")
                nc.tensor.transpose(wt_ps, W_bf, ident[0:26, 0:26])
                nc.vector.memset(Wt_bf, 0.0)
                nc.vector.tensor_copy(Wt_bf[:, 0:26], wt_ps)
                nc.vector.tensor_copy(Wt_gold, wt_ps)
            if t >= 2:
                emit_gold(t - 2)
                emit_count(t - 2)
                emit_exp(t - 2)
            if t >= 3:
                emit_u(t - 2)
                emit_mult(t - 2)
            if t == T - 1:
                emit_em(t - 1)
                emit_exp(t - 1)
                emit_u(t - 1)
                emit_mult(t - 1)
            emit_copies(t, xt_p)

        # ---- epilogue: drain the pipeline (the T-2 DP step was pulled
        # into the last loop iteration). The final multiply writes straight
        # into the output tile.
        emit_em(T - 1)
        emit_exp(T - 1)
        emit_gold(T - 2)
        emit_count(T - 2)
        emit_u(T - 1)
        nc.vector.tensor_mul(
            comb[:, 0:256].bitcast(f32), u[T - 1], eem[T - 1]
        )
        emit_gold(T - 1)

        # ---- finale ----
        # em_score = <Wt, St>, tr_score = <Tr, C>
        Sw = fpool.tile([128, 26], f32)
        nc.vector.tensor_mul(Sw, St_ps, Wt_gold)
        nc.vector.tensor_reduce(
            out=comb[:, 256:257], in_=Sw, axis=mybir.AxisListType.X, op=OP.add
        )
        Cw = fpool.tile([26, 26], f32)
        nc.vector.tensor_mul(Cw, C_ps, Tr_sb)
        nc.vector.tensor_reduce(
            out=comb[0:26, 257:258], in_=Cw, axis=mybir.AxisListType.X, op=OP.add
        )

        nc.sync.dma_start(out=out_d, in_=comb)

    fixed = _legalize_waits(nc.to_json_bytes())
    nc.to_json_bytes = lambda: fixed  # shadow for all compile paths
    return nc


def kernel(feat_x: np.ndarray, input_y: np.ndarray, params: np.ndarray) -> np.ndarray:
    from concourse.bass_utils import run_bass_kernel_spmd

    if "nc" not in _CACHE:
        _CACHE["nc"] = build_program()
    nc = _CACHE["nc"]

    feat_x = np.ascontiguousarray(feat_x, dtype=np.float32)
    input_y = np.ascontiguousarray(input_y, dtype=np.int32)
    params = np.ascontiguousarray(params, dtype=np.float32)

    in_maps = []
    for m in range(NCORES):
        sl = slice(m * BC, (m + 1) * BC)
        in_maps.append({"x": feat_x[sl], "y": input_y[sl], "p": params})

    res = run_bass_kernel_spmd(
        nc, in_maps, core_ids=list(range(NCORES)), trace=TRACE
    )
    _CACHE["last_results"] = res

    em_sum = tr_sum = lz_sum = 0.0
    for m in range(NCORES):
        out = res.results[m]["out"].astype(np.float64)
        em_sum += out[:, 256].sum()
        tr_sum += out[:, 257].sum()
        for g in range(4):
            lz_sum += np.log(out[32 * g : 32 * g + 26, 0:256].sum(axis=0)).sum()
    lz_sum += B * T * C_CONST
    loss = -(em_sum + tr_sum - lz_sum) / B
    return np.float32(loss)
